# revision 26
# baseline (speedup 1.0000x reference)
"""LISTA / AtasiNet Trainium2 kernel.

Math (reference): K=10 iterations of
    Z     = gamma - D @ (beta_k * W)          # [B,N]
    theta = mu_k / (|Z| + EPS)
    gamma = sign(Z) * max(|Z| - theta, 0)
    D     = gamma @ A.T - y                   # [B,M]

Distribution: pure data-parallel over batch B=2048 across 8 NeuronCores
(B_local=256 per core); A, W, mu, beta replicated. No collectives.

Per-core layout is fully transposed (batch on the free axis):
    gammaT [N=4096, B=256]   DT [M=1024, B=256]
so both matmuls take naturally-laid-out weights:
    Z^T = gammaT + (-W)^T @ (beta_k D)^T : lhsT = -W tile  [m128, n128]
    D^T = A @ gammaT                     : lhsT = A.T tile [n128, m128]
A.T and -W are precomputed on host; matmul inputs are bf16 with f32 PSUM
accumulation. The gamma subtraction is folded into the first matmul's
PSUM accumulation group as an identity matmul of gammaT (bf16 state), so
Z materializes directly in PSUM and the vector engine never does the
subtract.

Iteration k=0 is folded out analytically: for any mu_0 >= 0 the first
iteration yields gamma=0, D=-y. Device loop runs k=1..9; the last
iteration skips the (unused) D update and writes gamma out in f32.

Elementwise threshold uses the multiplicative form
    gamma = Z * relu(1 - mu_k / (|Z| (|Z|+EPS)))
with |Z|(|Z|+EPS) computed as Square(|Z| + EPS/2) - EPS^2/4 on ScalarE
and a fast approximate reciprocal (~51 ULP) on VectorE.
"""

import sys

for _p in ("/opt/trn_rl_repo",):
    if _p not in sys.path:
        sys.path.insert(0, _p)

import numpy as np
import ml_dtypes

import concourse.bass as bass
import concourse.mybir as mybir
import concourse.tile as tile
from concourse import bacc
from concourse.bass_utils import run_bass_kernel_spmd
from concourse.masks import make_identity

B, M, N, K = 2048, 1024, 4096, 10
EPS = 0.01
NCORES = 8
BL = B // NCORES            # 256 batch rows per core
P = 128
MT = M // P                 # 8 m-tiles
NT = N // P                 # 32 n-tiles
MM2_DELAY = 4               # emission lag of matmul2 behind matmul1 (in nt units)
MM2_DELAY_K1 = 2            # same, for the DMA-paced first iteration
BETAY_ENGINE = "gpsimd"     # engine for the per-iter beta*y prep
SPLIT_W0 = False            # split first W strip into two DMAs
THREE_DMA_QUEUES = False    # AT on the scalar engine queue
EW_BUFS = 4
PS1_BUFS = 4
PADJ_ENGINE = "vector"      # which engine runs the p-adjust tensor_scalar

F32 = mybir.dt.float32
BF16 = mybir.dt.bfloat16
ALU = mybir.AluOpType
ACT = mybir.ActivationFunctionType


# ---- custom fused DVE ops ----
# ABS_SHRINK_P: p = max(|Z| * (|Z| + C0), C1)       (C0=EPS, C1=floor)
# RELU_AFF_MUL: out = relu(Src0*C0 + C1) * Src1     (C0=-mu, C1=1.0, Src1=Z)
from concourse import dve_ops as _dvo
from concourse.dve_spec import Spec as _Spec, Src0 as _S0, Src1 as _S1, \
    C0 as _C0, C1 as _C1, Zero as _Z0, relu as _relu, maxx as _maxx

def _register(name, spec):
    """Reserve an opcode row, discover the uops sha, register the op."""
    import re as _re
    if name in _dvo._SUB_OPCODE_FOR_NAME:
        return next(op for op in _dvo.OPS if op.name == name)
    row = _dvo._CUSTOM_DVE_ROW_BASE + len(_dvo.OPS)
    assert row < 0x20
    _dvo._SUB_OPCODE_FOR_NAME[name] = row
    shas = {}
    for ver in ("v3",):
        try:
            _dvo.DveOp(name, spec, subdim=False, uops_sha={}).compile(ver)
        except ValueError as e:
            m = _re.search(r"%s: ([0-9a-f]+)" % ver, str(e))
            if not m:
                raise
            shas[ver] = m.group(1)
    op = _dvo.DveOp(name, spec, subdim=False, uops_sha=shas)
    _dvo.OPS.append(op)
    _dvo.CUSTOM_DVE_SPECS[name] = spec
    return op

_absu = _maxx(_S0, _Z0 - _S0)
ABS_SHRINK_P = _register("ABS_SHRINK_P_ATASI", _Spec(
    body=_maxx(_absu * (_absu + _C0), _C1),
    reference=lambda in0, in1, s0, s1, imm2: np.maximum(
        np.abs(in0.astype(np.float32)) * (np.abs(in0.astype(np.float32)) + s0), s1),
))
RELU_AFF_MUL = _register("RELU_AFF_MUL_ATASI", _Spec(
    body=_relu(_S0 * _C0 + _C1) * _S1,
    reference=lambda in0, in1, s0, s1, imm2: np.maximum(
        in0.astype(np.float32) * s0 + s1, 0.0) * in1,
))

_cached_nc = None


def build(n_iters=K - 1, debug_dt=False, ncores=NCORES):
    nc = bacc.Bacc("TRN2", target_bir_lowering=False, debug=False, num_devices=ncores)

    w_d = nc.dram_tensor("Wneg", [NT, M, P], BF16, kind="ExternalInput")
    dt0_d = nc.dram_tensor("dt0", [M, BL], BF16, kind="ExternalInput")
    at_d = nc.dram_tensor("AT", [N, M], BF16, kind="ExternalInput")
    yneg_d = nc.dram_tensor("ynegT", [M, BL], BF16, kind="ExternalInput")
    negmu_d = nc.dram_tensor("negmu", [P, K], F32, kind="ExternalInput")
    beta_d = nc.dram_tensor("beta", [P, K], F32, kind="ExternalInput")
    out_d = nc.dram_tensor("out", [N, BL], F32, kind="ExternalOutput")
    dt_d = (nc.dram_tensor("dt_out", [M, BL], BF16, kind="ExternalOutput")
            if debug_dt else None)

    with tile.TileContext(nc) as tc:
        with (
            tc.tile_pool(name="const", bufs=1) as cpool,
            tc.tile_pool(name="ew", bufs=EW_BUFS) as ew,
            tc.tile_pool(name="ps1", bufs=PS1_BUFS, space="PSUM") as ps1,
            tc.tile_pool(name="ps2", bufs=1, space="PSUM") as ps2,
        ):
            wsb = cpool.tile([P, NT, MT, P], BF16, tag="wsb")
            atsb = cpool.tile([P, NT, M], BF16, tag="atsb")
            ynegsb = cpool.tile([P, MT, BL], BF16, tag="ynegsb")
            negmu = cpool.tile([P, K], F32, tag="negmu")
            betasb = cpool.tile([P, K], F32, tag="betasb")
            gb = cpool.tile([P, NT, BL], BF16, tag="gb")        # gamma state (bf16)
            dt = cpool.tile([P, MT, BL], BF16, tag="dt")        # beta_k * D^T
            betay = cpool.tile([P, MT, BL], BF16, tag="betay")  # beta_{k+1} * (-y^T)
            ident = cpool.tile([P, P], BF16, tag="ident")

            halfeps = cpool.tile([P, 1], F32, tag="halfeps")
            nc.vector.memset(halfeps[:], EPS / 2)
            make_identity(nc, ident[:])

            # first matmul needs dt0 + W strip 0; lead with those on
            # separate queues, defer everything not needed until later.
            if SPLIT_W0:
                nc.gpsimd.dma_start(
                    wsb[:, 0, 0:4],
                    w_d.ap()[0, 0:4 * P].rearrange("(o p) c -> p o c", p=P),
                )
                nc.sync.dma_start(
                    wsb[:, 0, 4:8],
                    w_d.ap()[0, 4 * P:8 * P].rearrange("(o p) c -> p o c", p=P),
                )
            else:
                nc.gpsimd.dma_start(
                    wsb[:, 0], w_d.ap()[0].rearrange("(o p) c -> p o c", p=P)
                )
            nc.sync.dma_start(dt[:], dt0_d.ap().rearrange("(o p) b -> p o b", p=P))
            nc.scalar.dma_start(negmu[:], negmu_d[:])
            nc.scalar.dma_start(betasb[:], beta_d[:])
            dma_engines = [nc.sync, nc.gpsimd]
            if THREE_DMA_QUEUES:
                for nt in range(1, NT):
                    dma_engines[nt % 2].dma_start(
                        wsb[:, nt], w_d.ap()[nt].rearrange("(o p) c -> p o c", p=P)
                    )
                    nc.scalar.dma_start(
                        atsb[:, nt - 1], at_d.ap()[(nt - 1) * P:nt * P, :]
                    )
                nc.scalar.dma_start(atsb[:, NT - 1], at_d.ap()[(NT - 1) * P:NT * P, :])
            else:
                for nt in range(1, NT):
                    dma_engines[nt % 2].dma_start(
                        wsb[:, nt], w_d.ap()[nt].rearrange("(o p) c -> p o c", p=P)
                    )
                    dma_engines[(nt + 1) % 2].dma_start(
                        atsb[:, nt - 1], at_d.ap()[(nt - 1) * P:nt * P, :]
                    )
                nc.sync.dma_start(atsb[:, NT - 1], at_d.ap()[(NT - 1) * P:NT * P, :])
            nc.scalar.dma_start(
                ynegsb[:], yneg_d.ap().rearrange("(o p) b -> p o b", p=P)
            )

            out_v = out_d.ap().rearrange("(o p) b -> p o b", p=P)

            def emit_mm2(k, nt):
                """matmul2: accumulate A@gammaT for this nt into all 8 D slices.

                Two m-slices share one PSUM bank; start=True clears the WHOLE
                bank, so only the first (even) slice of each pair may set it.
                The odd slice's first write lands on cleared has_written bits
                and overwrites anyway."""
                for mt in range(MT):
                    dacc = dpsum[mt // 2][:, (mt % 2) * BL:(mt % 2) * BL + BL]
                    nc.tensor.matmul(
                        dacc,
                        atsb[:, nt, mt * P:(mt + 1) * P],
                        gb[:, nt],
                        start=(nt == 0 and mt % 2 == 0),
                        stop=(nt == NT - 1),
                        skip_group_check=True,
                    )

            for k in range(1, 1 + n_iters):
                last = (k == n_iters)
                if last and debug_dt:
                    last = False  # still run the D update so we can dump it
                if not last:
                    # prep beta_{k+1} * (-y)^T for the D epilogue
                    for mt in range(MT):
                        getattr(nc, BETAY_ENGINE).tensor_scalar_mul(
                            betay[:, mt], ynegsb[:, mt], betasb[:, k + 1:k + 2]
                        )
                    # paired D accumulators: 4 banks x [128, 512] hold 8 m-slices
                    dpsum = [
                        ps2.tile([P, 2 * BL], F32, tag=f"dps{j}", name=f"dps{j}_{k}")
                        for j in range(4)
                    ]

                delay = MM2_DELAY_K1 if k == 1 else MM2_DELAY
                for nt in range(NT):
                    # Z^T accumulates in PSUM: sum_m (-W)^T (beta D)^T + I gamma^T
                    z = ps1.tile([P, BL], F32, tag="z")
                    for mt in range(MT):
                        nc.tensor.matmul(
                            z,
                            wsb[:, nt, mt],
                            dt[:, mt],
                            start=(mt == 0),
                            stop=(mt == MT - 1 and k == 1),
                            skip_group_check=True,
                        )
                    if k > 1:
                        # gamma - beta*D@W via identity matmul (gamma=0 at k=1)
                        nc.tensor.matmul(
                            z, ident[:], gb[:, nt],
                            start=False, stop=True, skip_group_check=True,
                        )
                    pp = ew.tile([P, BL], F32, tag="pp")
                    nc.vector._custom_dve(
                        ABS_SHRINK_P, out=pp, in0=z, s0=EPS, s1=1e-30)
                    r = ew.tile([P, BL], F32, tag="r")
                    nc.vector.reciprocal_approx_fast(r, pp)
                    # gamma = relu(1 - mu/p) * Z
                    if last:
                        gstage = ew.tile([P, BL], F32, tag="gstage")
                        nc.vector._custom_dve(
                            RELU_AFF_MUL, out=gstage, in0=r, in1=z,
                            s0=negmu[:, k:k + 1], s1=1.0)
                        dma_engines[nt % 2].dma_start(out_v[:, nt], gstage[:])
                    else:
                        nc.vector._custom_dve(
                            RELU_AFF_MUL, out=gb[:, nt], in0=r, in1=z,
                            s0=negmu[:, k:k + 1], s1=1.0)
                        if nt >= delay:
                            emit_mm2(k, nt - delay)

                if not last:
                    for nt in range(NT - delay, NT):
                        emit_mm2(k, nt)
                    # D epilogue: DT = beta_{k+1}*P + beta_{k+1}*(-y),
                    # one op per PSUM bank (two m-slices at FD=512)
                    for j in range(0, 4):
                        nc.vector.scalar_tensor_tensor(
                            dt[:, 2 * j:2 * j + 2], dpsum[j][:],
                            betasb[:, k + 1:k + 2], betay[:, 2 * j:2 * j + 2],
                            ALU.mult, ALU.add,
                        )

            if debug_dt:
                for nt in range(NT):
                    gstage2 = ew.tile([P, BL], F32, tag="gstage2", name=f"gs2_{nt}")
                    nc.vector.tensor_copy(out=gstage2[:], in_=gb[:, nt])
                    nc.sync.dma_start(out_v[:, nt], gstage2[:])
                nc.sync.dma_start(
                    dt_d.ap().rearrange("(o p) b -> p o b", p=P), dt[:]
                )

    nc.compile()
    return nc


def kernel(y, A, W, mu, beta):
    global _cached_nc
    y = np.asarray(y, np.float32)
    A = np.asarray(A, np.float32)
    W = np.asarray(W, np.float32)
    mu = np.asarray(mu, np.float32)
    beta = np.asarray(beta, np.float32)

    if _cached_nc is None:
        _cached_nc = build()
    nc = _cached_nc

    in_maps = make_in_maps(y, A, W, mu, beta)
    res = run_bass_kernel_spmd(nc, in_maps, core_ids=list(range(NCORES)))
    # out per core: gammaT [N, BL] -> gather to [B, N]
    return np.concatenate([r["out"].T for r in res.results], axis=0)


def make_in_maps(y, A, W, mu, beta):
    wb = np.ascontiguousarray(
        (-W).astype(ml_dtypes.bfloat16).reshape(M, NT, P).transpose(1, 0, 2))
    atb = np.ascontiguousarray(A.T).astype(ml_dtypes.bfloat16)
    ynegt = np.ascontiguousarray(-y.T).astype(ml_dtypes.bfloat16)  # [M, B]
    negmu_b = np.ascontiguousarray(np.broadcast_to(-mu, (P, K))).astype(np.float32)
    beta_b = np.ascontiguousarray(np.broadcast_to(beta, (P, K))).astype(np.float32)
    in_maps = []
    for c in range(NCORES):
        yc = np.ascontiguousarray(ynegt[:, c * BL:(c + 1) * BL])
        in_maps.append({
            "Wneg": wb,
            "AT": atb,
            "ynegT": yc,
            "dt0": (yc.astype(np.float32) * beta[1]).astype(ml_dtypes.bfloat16),
            "negmu": negmu_b,
            "beta": beta_b,
        })
    return in_maps


if __name__ == "__main__":
    rng = np.random.default_rng(0)
    y = rng.standard_normal((B, M)).astype(np.float32)
    A = (rng.standard_normal((M, N)) / np.sqrt(M)).astype(np.float32)
    W = (rng.standard_normal((M, N)) / np.sqrt(M)).astype(np.float32)
    mu = rng.random(K).astype(np.float32)
    beta = rng.random(K).astype(np.float32)
    g = kernel(y=y, A=A, W=W, mu=mu, beta=beta)
    print("out", g.shape, g.dtype, np.abs(g).max())


# revision 29
# speedup vs baseline: 1.2832x; 1.2832x over previous
"""LISTA / AtasiNet Trainium2 kernel.

Math (reference): K=10 iterations of
    Z     = gamma - D @ (beta_k * W)          # [B,N]
    theta = mu_k / (|Z| + EPS)
    gamma = sign(Z) * max(|Z| - theta, 0)
    D     = gamma @ A.T - y                   # [B,M]

Distribution: pure data-parallel over batch B=2048 across 8 NeuronCores
(B_local=256 per core); A, W, mu, beta replicated. No collectives.

Per-core layout is fully transposed (batch on the free axis):
    gammaT [N=4096, B=256]   DT [M=1024, B=256]
so both matmuls take naturally-laid-out weights:
    Z^T = gammaT + (-W)^T @ (beta_k D)^T : lhsT = -W tile  [m128, n128]
    D^T = A @ gammaT                     : lhsT = A.T tile [n128, m128]
A.T and -W are precomputed on host; matmul inputs are bf16 with f32 PSUM
accumulation. The gamma subtraction is folded into the first matmul's
PSUM accumulation group as an identity matmul of gammaT (bf16 state), so
Z materializes directly in PSUM and the vector engine never does the
subtract.

Iteration k=0 is folded out analytically: for any mu_0 >= 0 the first
iteration yields gamma=0, D=-y. Device loop runs k=1..9; the last
iteration skips the (unused) D update and writes gamma out in f32.

Elementwise threshold uses the multiplicative form
    gamma = Z * relu(1 - mu_k / (|Z| (|Z|+EPS)))
with |Z|(|Z|+EPS) computed as Square(|Z| + EPS/2) - EPS^2/4 on ScalarE
and a fast approximate reciprocal (~51 ULP) on VectorE.
"""

import sys

for _p in ("/opt/trn_rl_repo",):
    if _p not in sys.path:
        sys.path.insert(0, _p)

import numpy as np
import ml_dtypes

import concourse.bass as bass
import concourse.mybir as mybir
import concourse.tile as tile
from concourse import bacc
from concourse.bass_utils import run_bass_kernel_spmd
from concourse.masks import make_identity

B, M, N, K = 2048, 1024, 4096, 10
EPS = 0.01
NCORES = 8
BL = B // NCORES            # 256 batch rows per core
P = 128
MT = M // P                 # 8 m-tiles
NT = N // P                 # 32 n-tiles
MM2_DELAY = 4               # emission lag of matmul2 behind matmul1 (in nt units)
MM2_DELAY_K1 = 2            # same, for the DMA-paced first iteration
BETAY_ENGINE = "gpsimd"     # engine for the per-iter beta*y prep
SPLIT_W0 = False            # split first W strip into two DMAs
THREE_DMA_QUEUES = False    # AT on the scalar engine queue
LAST_ITER_ACT_CHAIN = True  # ScalarE-based |Z|(|Z|+eps) in the final iteration
EW_BUFS = 4
PS1_BUFS = 4
PADJ_ENGINE = "vector"      # which engine runs the p-adjust tensor_scalar

F32 = mybir.dt.float32
BF16 = mybir.dt.bfloat16
ALU = mybir.AluOpType
ACT = mybir.ActivationFunctionType


# ---- custom fused DVE ops ----
# ABS_SHRINK_P: p = max(|Z| * (|Z| + C0), C1)       (C0=EPS, C1=floor)
# RELU_AFF_MUL: out = relu(Src0*C0 + C1) * Src1     (C0=-mu, C1=1.0, Src1=Z)
from concourse import dve_ops as _dvo
from concourse.dve_spec import Spec as _Spec, Src0 as _S0, Src1 as _S1, \
    C0 as _C0, C1 as _C1, Zero as _Z0, relu as _relu, maxx as _maxx

def _register(name, spec):
    """Reserve an opcode row, discover the uops sha, register the op."""
    import re as _re
    if name in _dvo._SUB_OPCODE_FOR_NAME:
        return next(op for op in _dvo.OPS if op.name == name)
    row = _dvo._CUSTOM_DVE_ROW_BASE + len(_dvo.OPS)
    assert row < 0x20
    _dvo._SUB_OPCODE_FOR_NAME[name] = row
    shas = {}
    for ver in ("v3",):
        try:
            _dvo.DveOp(name, spec, subdim=False, uops_sha={}).compile(ver)
        except ValueError as e:
            m = _re.search(r"%s: ([0-9a-f]+)" % ver, str(e))
            if not m:
                raise
            shas[ver] = m.group(1)
    op = _dvo.DveOp(name, spec, subdim=False, uops_sha=shas)
    _dvo.OPS.append(op)
    _dvo.CUSTOM_DVE_SPECS[name] = spec
    return op

_absu = _maxx(_S0, _Z0 - _S0)
ABS_SHRINK_P = _register("ABS_SHRINK_P_ATASI", _Spec(
    body=_maxx(_absu * (_absu + _C0), _C1),
    reference=lambda in0, in1, s0, s1, imm2: np.maximum(
        np.abs(in0.astype(np.float32)) * (np.abs(in0.astype(np.float32)) + s0), s1),
))
RELU_AFF_MUL = _register("RELU_AFF_MUL_ATASI", _Spec(
    body=_relu(_S0 * _C0 + _C1) * _S1,
    reference=lambda in0, in1, s0, s1, imm2: np.maximum(
        in0.astype(np.float32) * s0 + s1, 0.0) * in1,
))

_cached_nc = None


def build(n_iters=K - 1, debug_dt=False, ncores=NCORES):
    nc = bacc.Bacc("TRN2", target_bir_lowering=False, debug=False, num_devices=ncores)

    w_d = nc.dram_tensor("Wneg", [NT, M, P], BF16, kind="ExternalInput")
    dt0_d = nc.dram_tensor("dt0", [M, BL], BF16, kind="ExternalInput")
    at_d = nc.dram_tensor("AT", [N, M], BF16, kind="ExternalInput")
    yneg_d = nc.dram_tensor("ynegT", [M, BL], BF16, kind="ExternalInput")
    negmu_d = nc.dram_tensor("negmu", [P, K], F32, kind="ExternalInput")
    beta_d = nc.dram_tensor("beta", [P, K], F32, kind="ExternalInput")
    out_d = nc.dram_tensor("out", [N, BL], F32, kind="ExternalOutput")
    dt_d = (nc.dram_tensor("dt_out", [M, BL], BF16, kind="ExternalOutput")
            if debug_dt else None)

    with tile.TileContext(nc) as tc:
        with (
            tc.tile_pool(name="const", bufs=1) as cpool,
            tc.tile_pool(name="ew", bufs=EW_BUFS) as ew,
            tc.tile_pool(name="ps1", bufs=PS1_BUFS, space="PSUM") as ps1,
            tc.tile_pool(name="ps2", bufs=1, space="PSUM") as ps2,
        ):
            wsb = cpool.tile([P, NT, MT, P], BF16, tag="wsb")
            atsb = cpool.tile([P, NT, M], BF16, tag="atsb")
            ynegsb = cpool.tile([P, MT, BL], BF16, tag="ynegsb")
            negmu = cpool.tile([P, K], F32, tag="negmu")
            betasb = cpool.tile([P, K], F32, tag="betasb")
            gb = cpool.tile([P, NT, BL], BF16, tag="gb")        # gamma state (bf16)
            dt = cpool.tile([P, MT, BL], BF16, tag="dt")        # beta_k * D^T
            betay = cpool.tile([P, MT, BL], BF16, tag="betay")  # beta_{k+1} * (-y^T)
            ident = cpool.tile([P, P], BF16, tag="ident")

            # first matmul needs dt0 + W strip 0; lead with those on
            # separate queues, defer everything not needed until later.
            nc.sync.dma_start(
                wsb[:, 0], w_d.ap()[0].rearrange("(o p) c -> p o c", p=P)
            )
            nc.gpsimd.dma_start(dt[:], dt0_d.ap().rearrange("(o p) b -> p o b", p=P))
            nc.scalar.dma_start(negmu[:], negmu_d[:])
            nc.scalar.dma_start(betasb[:], beta_d[:])

            halfeps = cpool.tile([P, 1], F32, tag="halfeps")
            nc.vector.memset(halfeps[:], EPS / 2)
            make_identity(nc, ident[:])
            dma_engines = [nc.sync, nc.gpsimd]
            if THREE_DMA_QUEUES:
                for nt in range(1, NT):
                    dma_engines[nt % 2].dma_start(
                        wsb[:, nt], w_d.ap()[nt].rearrange("(o p) c -> p o c", p=P)
                    )
                    nc.scalar.dma_start(
                        atsb[:, nt - 1], at_d.ap()[(nt - 1) * P:nt * P, :]
                    )
                nc.scalar.dma_start(atsb[:, NT - 1], at_d.ap()[(NT - 1) * P:NT * P, :])
            else:
                for nt in range(1, NT):
                    dma_engines[nt % 2].dma_start(
                        wsb[:, nt], w_d.ap()[nt].rearrange("(o p) c -> p o c", p=P)
                    )
                    dma_engines[(nt + 1) % 2].dma_start(
                        atsb[:, nt - 1], at_d.ap()[(nt - 1) * P:nt * P, :]
                    )
                nc.sync.dma_start(atsb[:, NT - 1], at_d.ap()[(NT - 1) * P:NT * P, :])
            nc.scalar.dma_start(
                ynegsb[:], yneg_d.ap().rearrange("(o p) b -> p o b", p=P)
            )

            out_v = out_d.ap().rearrange("(o p) b -> p o b", p=P)

            def emit_mm2(k, nt):
                """matmul2: accumulate A@gammaT for this nt into all 8 D slices.

                Two m-slices share one PSUM bank; start=True clears the WHOLE
                bank, so only the first (even) slice of each pair may set it.
                The odd slice's first write lands on cleared has_written bits
                and overwrites anyway."""
                for mt in range(MT):
                    dacc = dpsum[mt // 2][:, (mt % 2) * BL:(mt % 2) * BL + BL]
                    nc.tensor.matmul(
                        dacc,
                        atsb[:, nt, mt * P:(mt + 1) * P],
                        gb[:, nt],
                        start=(nt == 0 and mt % 2 == 0),
                        stop=(nt == NT - 1),
                        skip_group_check=True,
                    )

            for k in range(1, 1 + n_iters):
                last = (k == n_iters)
                if last and debug_dt:
                    last = False  # still run the D update so we can dump it
                if not last:
                    # prep beta_{k+1} * (-y)^T for the D epilogue
                    for mt in range(MT):
                        getattr(nc, BETAY_ENGINE).tensor_scalar_mul(
                            betay[:, mt], ynegsb[:, mt], betasb[:, k + 1:k + 2]
                        )
                    # paired D accumulators: 4 banks x [128, 512] hold 8 m-slices
                    dpsum = [
                        ps2.tile([P, 2 * BL], F32, tag=f"dps{j}", name=f"dps{j}_{k}")
                        for j in range(4)
                    ]

                delay = MM2_DELAY_K1 if k == 1 else MM2_DELAY
                for nt in range(NT):
                    # Z^T accumulates in PSUM: sum_m (-W)^T (beta D)^T + I gamma^T
                    z = ps1.tile([P, BL], F32, tag="z")
                    for mt in range(MT):
                        nc.tensor.matmul(
                            z,
                            wsb[:, nt, mt],
                            dt[:, mt],
                            start=(mt == 0),
                            stop=(mt == MT - 1 and k == 1),
                            skip_group_check=True,
                        )
                    if k > 1:
                        # gamma - beta*D@W via identity matmul (gamma=0 at k=1)
                        nc.tensor.matmul(
                            z, ident[:], gb[:, nt],
                            start=False, stop=True, skip_group_check=True,
                        )
                    pp = ew.tile([P, BL], F32, tag="pp")
                    if last and LAST_ITER_ACT_CHAIN:
                        # no mm2 in the last iteration, so DVE (not PE) paces
                        # it; compute p on ScalarE to rebalance.
                        az = ew.tile([P, BL], F32, tag="az")
                        nc.scalar.activation(az, z, ACT.Abs)
                        nc.scalar.activation(pp, az, ACT.Square,
                                             bias=halfeps[:, 0:1])
                        nc.vector.tensor_scalar(
                            pp, pp, -EPS * EPS / 4, 1e-30, ALU.add, ALU.max)
                    else:
                        nc.vector._custom_dve(
                            ABS_SHRINK_P, out=pp, in0=z, s0=EPS, s1=1e-30)
                    r = ew.tile([P, BL], F32, tag="r")
                    nc.vector.reciprocal_approx_fast(r, pp)
                    # gamma = relu(1 - mu/p) * Z
                    if last:
                        gstage = ew.tile([P, BL], F32, tag="gstage")
                        nc.vector._custom_dve(
                            RELU_AFF_MUL, out=gstage, in0=r, in1=z,
                            s0=negmu[:, k:k + 1], s1=1.0)
                        dma_engines[nt % 2].dma_start(out_v[:, nt], gstage[:])
                    else:
                        nc.vector._custom_dve(
                            RELU_AFF_MUL, out=gb[:, nt], in0=r, in1=z,
                            s0=negmu[:, k:k + 1], s1=1.0)
                        if nt >= delay:
                            emit_mm2(k, nt - delay)

                if not last:
                    for nt in range(NT - delay, NT):
                        emit_mm2(k, nt)
                    # D epilogue: DT = beta_{k+1}*P + beta_{k+1}*(-y),
                    # one op per PSUM bank (two m-slices at FD=512)
                    for j in range(0, 4):
                        nc.vector.scalar_tensor_tensor(
                            dt[:, 2 * j:2 * j + 2], dpsum[j][:],
                            betasb[:, k + 1:k + 2], betay[:, 2 * j:2 * j + 2],
                            ALU.mult, ALU.add,
                        )

            if debug_dt:
                for nt in range(NT):
                    gstage2 = ew.tile([P, BL], F32, tag="gstage2", name=f"gs2_{nt}")
                    nc.vector.tensor_copy(out=gstage2[:], in_=gb[:, nt])
                    nc.sync.dma_start(out_v[:, nt], gstage2[:])
                nc.sync.dma_start(
                    dt_d.ap().rearrange("(o p) b -> p o b", p=P), dt[:]
                )

    nc.compile()
    return nc


def kernel(y, A, W, mu, beta):
    global _cached_nc
    y = np.asarray(y, np.float32)
    A = np.asarray(A, np.float32)
    W = np.asarray(W, np.float32)
    mu = np.asarray(mu, np.float32)
    beta = np.asarray(beta, np.float32)

    if _cached_nc is None:
        _cached_nc = build()
    nc = _cached_nc

    in_maps = make_in_maps(y, A, W, mu, beta)
    res = run_bass_kernel_spmd(nc, in_maps, core_ids=list(range(NCORES)))
    # out per core: gammaT [N, BL] -> gather to [B, N]
    return np.concatenate([r["out"].T for r in res.results], axis=0)


def make_in_maps(y, A, W, mu, beta):
    wb = np.ascontiguousarray(
        (-W).astype(ml_dtypes.bfloat16).reshape(M, NT, P).transpose(1, 0, 2))
    atb = np.ascontiguousarray(A.T).astype(ml_dtypes.bfloat16)
    ynegt = np.ascontiguousarray(-y.T).astype(ml_dtypes.bfloat16)  # [M, B]
    negmu_b = np.ascontiguousarray(np.broadcast_to(-mu, (P, K))).astype(np.float32)
    beta_b = np.ascontiguousarray(np.broadcast_to(beta, (P, K))).astype(np.float32)
    in_maps = []
    for c in range(NCORES):
        yc = np.ascontiguousarray(ynegt[:, c * BL:(c + 1) * BL])
        in_maps.append({
            "Wneg": wb,
            "AT": atb,
            "ynegT": yc,
            "dt0": (yc.astype(np.float32) * beta[1]).astype(ml_dtypes.bfloat16),
            "negmu": negmu_b,
            "beta": beta_b,
        })
    return in_maps


if __name__ == "__main__":
    rng = np.random.default_rng(0)
    y = rng.standard_normal((B, M)).astype(np.float32)
    A = (rng.standard_normal((M, N)) / np.sqrt(M)).astype(np.float32)
    W = (rng.standard_normal((M, N)) / np.sqrt(M)).astype(np.float32)
    mu = rng.random(K).astype(np.float32)
    beta = rng.random(K).astype(np.float32)
    g = kernel(y=y, A=A, W=W, mu=mu, beta=beta)
    print("out", g.shape, g.dtype, np.abs(g).max())
